# revision 32
# baseline (speedup 1.0000x reference)
"""Mixtral-style sparse MoE block on 8 Trainium2 NeuronCores.

Strategy (expert parallelism, per the sharding hint):
- The tiny gate MLP + softmax + top-2 routing runs on the host in fp64
  (routing decisions must match the fp32 reference; fp64 is the closest
  approximation to the true value and ~9 GFLOP is sub-second on host).
- Tokens are dispatched (gathered) per expert on the host; each of the 8
  NeuronCores holds exactly one expert's weights and runs the expert FFN
  y = (relu(x@w1) * (x@w3)) @ w2 for its gathered tokens.
- The top-2 combine weight is folded into the tokens before dispatch as
  sqrt(w): relu is positively homogeneous and the gated product is
  quadratic, so scaling x by sqrt(w) scales the FFN output by exactly w.
  This makes the device kernel a pure dense FFN with zero extra work.
- Matmuls run as float32r (fp32 with an 11-bit mantissa, processed at
  1 cycle/row by the PE vs 4 for full fp32). Operands are pre-rounded to
  f32r on the host; accumulation is fp32 in PSUM.
- Everything on the device is laid out transposed (feature dim on the
  128 SBUF partitions, tokens on the free axis) so all three matmuls use
  the weights as the stationary operand with no on-device transposes.
"""

import os

import numpy as np

import concourse.bacc as bacc
import concourse.mybir as mybir
import concourse.tile as tile
from concourse.bass_utils import run_bass_kernel_spmd

E = 8          # experts == cores
TOP_K = 2
H = 1024       # embed
F = 2048       # ffn
KO = H // 128  # 8 contraction chunks for H
NF = F // 128  # 16 f chunks
CMAX = 1280    # max tokens per expert per launch (SBUF budget)

F32 = mybir.dt.float32
F32R = mybir.dt.float32r

_PROGRAM_CACHE: dict[int, object] = {}
LAST_RESULTS = None  # BassKernelResults of the expert launch (for test harness)


def _ensure_profile_hook():
    """run_bass_kernel_spmd's trace path (BASS_TRACE=1) imports
    antenv.axon_hooks, which slim agent images lack. If it's missing,
    recreate the hook from trn_agent_boot's ctypes implementation and stub
    the artifact upload, so profiling works instead of crashing."""
    try:
        import antenv.axon_hooks  # noqa: F401
        return
    except ImportError:
        pass
    try:
        import sys
        import types

        import concourse.bass_utils as bu
        from trn_agent_boot.trn_boot import _ntff_profile_via_ctypes

        hook = _ntff_profile_via_ctypes("/opt/axon/libaxon_pjrt.so")
        mod = types.ModuleType("antenv.axon_hooks")
        mod.get_axon_ntff_profile_hook = lambda: hook
        sys.modules["antenv.axon_hooks"] = mod
        bu.upload_artifacts = lambda tmpdir: tmpdir
    except Exception:
        os.environ["BASS_NEVER_TRACE"] = "1"  # degrade: run untraced


def _round_f32r(x: np.ndarray) -> np.ndarray:
    """Round fp32 to f32r (round-to-nearest-even to 11 mantissa bits,
    low 12 bits zeroed) — matches what the PE requires for f32r operands."""
    u = np.ascontiguousarray(x, dtype=np.float32).view(np.uint32)
    lsb = (u >> np.uint32(12)) & np.uint32(1)
    r = (u + np.uint32(0x7FF) + lsb) & np.uint32(0xFFFFF000)
    return r.view(np.float32)


def _token_tiles(c: int):
    """Split the padded token dim into matmul tiles of 512 (tail >= 256).

    512-wide tiles keep the PE streaming above the ~190ns implicit f32r
    weight-load shadow; f32r also needs a moving dim >= 256 for full rate
    (even 448-wide tiles measured slower — weight-load-bound), so `c` is
    kept a multiple of 256 and only the tail tile drops to 256.
    """
    tiles = []
    c0 = 0
    while c0 < c:
        cw = 512 if c - c0 >= 512 else c - c0
        tiles.append((c0, cw))
        c0 += cw
    return tiles


def _build_expert_program(c: int):
    """Bass program: per-core expert FFN for `c` (transposed) tokens.

    Inputs (per core, all f32r pre-rounded on host):
      xt [H, c]             tokens, transposed, pre-scaled by sqrt(combine)
      w1 [NF, 128, KO, 128] w1 rearranged: [f, ki, ko, fi] = w1[ko*128+ki, f*128+fi]
      w3 [NF, 128, KO, 128] same layout as w1
      w2 [KO, 128, NF, 128] [h, fi, fo, hc] = w2[fo*128+fi, h*128+hc]
    Output:
      yt [H, c] fp32        transposed expert output (already combine-weighted)
    """
    nc = bacc.Bacc("TRN2", target_bir_lowering=False, debug=False, num_devices=E)
    xtd = nc.declare_dram_parameter("xt", [H, c], F32R, isOutput=False)
    w1d = nc.declare_dram_parameter("w1", [NF, 128, KO, 128], F32R, isOutput=False)
    w3d = nc.declare_dram_parameter("w3", [NF, 128, KO, 128], F32R, isOutput=False)
    w2d = nc.declare_dram_parameter("w2", [KO, 128, NF, 128], F32R, isOutput=False)
    ytd = nc.declare_dram_parameter("yt", [H, c], F32, isOutput=True)

    ttiles = _token_tiles(c)

    with tile.TileContext(nc) as tc:
        with (
            tc.tile_pool(name="xtp", bufs=1) as xtp,
            tc.tile_pool(name="wp", bufs=2) as wp,
            tc.tile_pool(name="hp", bufs=1) as hp,
            tc.tile_pool(name="op", bufs=4) as op,
            tc.tile_pool(name="pp", bufs=2, space="PSUM") as pp,
        ):
            # The first f-chunk's weights are queued before the xt stream so
            # the PE's first accumulation group isn't stuck behind the whole
            # 5MB token load; xt itself loads per (token-tile, k) so early
            # matmuls only depend on the slices they actually read.
            w1t0 = wp.tile([128, KO, 128], F32R, tag="w1t", name="w1t0")
            w3t0 = wp.tile([128, KO, 128], F32R, tag="w3t", name="w3t0")
            xts = [xtp.tile([128, c], F32R, tag=f"xt{k}", name=f"xt{k}")
                   for k in range(KO)]
            # Phase A walks token tiles narrowest-first: the first matmul
            # group then only needs w1[0] + the smallest xt slice set.
            ttiles_a = sorted(ttiles, key=lambda t: t[1])
            nc.sync.dma_start(out=w1t0[:], in_=w1d[0])
            for ti, (c0, cw) in enumerate(ttiles_a):
                for k in range(KO):
                    nc.sync.dma_start(
                        out=xts[k][:, c0:c0 + cw],
                        in_=xtd[k * 128:(k + 1) * 128, c0:c0 + cw],
                    )
                if ti == 0:
                    nc.sync.dma_start(out=w3t0[:], in_=w3d[0])

            hts = [
                hp.tile([128, c], F32R, tag=f"ht{f}", name=f"ht{f}")
                for f in range(NF)
            ]

            # Phase A: hT[f] = relu(w1[f]^T @ xt) * (w3[f]^T @ xt)
            for f in range(NF):
                if f == 0:
                    w1t, w3t = w1t0, w3t0
                else:
                    w1t = wp.tile([128, KO, 128], F32R, tag="w1t", name=f"w1t{f}")
                    nc.sync.dma_start(out=w1t[:], in_=w1d[f])
                    w3t = wp.tile([128, KO, 128], F32R, tag="w3t", name=f"w3t{f}")
                    nc.sync.dma_start(out=w3t[:], in_=w3d[f])
                for (c0, cw) in ttiles_a:
                    p1 = pp.tile([128, 512], F32, tag="p1", name="p1")
                    for k in range(KO):
                        nc.tensor.matmul(
                            p1[:, :cw], w1t[:, k, :], xts[k][:, c0:c0 + cw],
                            start=(k == 0), stop=(k == KO - 1),
                        )
                    p3 = pp.tile([128, 512], F32, tag="p3", name="p3")
                    for k in range(KO):
                        nc.tensor.matmul(
                            p3[:, :cw], w3t[:, k, :], xts[k][:, c0:c0 + cw],
                            start=(k == 0), stop=(k == KO - 1),
                        )
                    rl = op.tile([128, 512], F32, tag="rl", name="rl")
                    nc.scalar.activation(
                        rl[:, :cw], p1[:, :cw], mybir.ActivationFunctionType.Relu
                    )
                    nc.vector.tensor_mul(
                        hts[f][:, c0:c0 + cw], rl[:, :cw], p3[:, :cw]
                    )

            # Phase B: yt[h] = sum_f w2[h][f]^T @ hT[f]
            for h in range(KO):
                w2t = wp.tile([128, NF, 128], F32R, tag="w2t", name=f"w2t{h}")
                nc.sync.dma_start(out=w2t[:], in_=w2d[h])
                for (c0, cw) in ttiles:
                    po = pp.tile([128, 512], F32, tag="po", name="po")
                    for f in range(NF):
                        nc.tensor.matmul(
                            po[:, :cw], w2t[:, f, :], hts[f][:, c0:c0 + cw],
                            start=(f == 0), stop=(f == NF - 1),
                        )
                    yo = op.tile([128, 512], F32, tag="yo", name="yo")
                    nc.vector.tensor_copy(yo[:, :cw], po[:, :cw])
                    nc.sync.dma_start(
                        out=ytd[h * 128:(h + 1) * 128, c0:c0 + cw], in_=yo[:, :cw]
                    )

    nc.compile()
    return nc


def _gate_and_route(x, gw1, gb1, gw2, gb2):
    """Host gate MLP (fp64) + softmax + top-2 routing."""
    g = x.astype(np.float64) @ gw1.astype(np.float64) + gb1.astype(np.float64)
    g = np.where(g > 0.0, g, np.expm1(np.minimum(g, 0.0)))
    logits = g @ gw2.astype(np.float64) + gb2.astype(np.float64)
    z = logits - logits.max(axis=-1, keepdims=True)
    p = np.exp(z)
    p /= p.sum(axis=-1, keepdims=True)
    sel = np.argsort(-p, axis=-1, kind="stable")[:, :TOP_K]  # ties: lowest idx
    rw = np.take_along_axis(p, sel, axis=-1)
    rw = rw / rw.sum(axis=-1, keepdims=True)
    return logits.astype(np.float32), sel, rw


def kernel(hidden_states, gw1, gb1, gw2, gb2, w1, w3, w2):
    global LAST_RESULTS
    _ensure_profile_hook()
    hidden_states = np.asarray(hidden_states, dtype=np.float32)
    gw1 = np.asarray(gw1, dtype=np.float32)
    gb1 = np.asarray(gb1, dtype=np.float32)
    gw2 = np.asarray(gw2, dtype=np.float32)
    gb2 = np.asarray(gb2, dtype=np.float32)
    w1 = np.asarray(w1, dtype=np.float32)
    w3 = np.asarray(w3, dtype=np.float32)
    w2 = np.asarray(w2, dtype=np.float32)

    B, S, _ = hidden_states.shape
    x = hidden_states.reshape(-1, H)
    t = x.shape[0]

    logits, sel, rw = _gate_and_route(x, gw1, gb1, gw2, gb2)

    # Dispatch: token indices + sqrt(combine) scale per expert.
    idx_e, scl_e = [], []
    for e in range(E):
        tok, slot = np.nonzero(sel == e)
        idx_e.append(tok)
        scl_e.append(np.sqrt(rw[tok, slot]).astype(np.float32))
    counts = np.array([len(i) for i in idx_e])

    # Uniform capacity (SPMD: same program on all 8 cores), multiple of 256.
    max_cnt = int(counts.max())
    n_slabs = max(1, -(-max_cnt // CMAX))  # ceil
    slab_cnt = -(-max_cnt // n_slabs)
    c = max(512, -(-slab_cnt // 256) * 256)

    if c not in _PROGRAM_CACHE:
        _PROGRAM_CACHE[c] = _build_expert_program(c)
    nc = _PROGRAM_CACHE[c]

    # Per-expert weight repack (f32r, layouts documented in _build_expert_program)
    w1r = _round_f32r(
        np.ascontiguousarray(w1.reshape(E, KO, 128, NF, 128).transpose(0, 3, 2, 1, 4))
    )
    w3r = _round_f32r(
        np.ascontiguousarray(w3.reshape(E, KO, 128, NF, 128).transpose(0, 3, 2, 1, 4))
    )
    w2r = _round_f32r(
        np.ascontiguousarray(w2.reshape(E, NF, 128, KO, 128).transpose(0, 3, 2, 1, 4))
    )

    out = np.zeros((t, H), dtype=np.float32)
    for slab in range(n_slabs):
        in_maps = []
        slab_idx = []
        for e in range(E):
            lo, hi = slab * slab_cnt, min((slab + 1) * slab_cnt, counts[e])
            idx = idx_e[e][lo:hi] if hi > lo else idx_e[e][:0]
            slab_idx.append(idx)
            xg = x[idx] * scl_e[e][lo:hi, None]  # [n, H]
            xt = np.zeros((H, c), dtype=np.float32)
            xt[:, :len(idx)] = xg.T
            in_maps.append({
                "xt": _round_f32r(xt),
                "w1": w1r[e],
                "w3": w3r[e],
                "w2": w2r[e],
            })
        res = run_bass_kernel_spmd(nc, in_maps, core_ids=list(range(E)))
        LAST_RESULTS = res
        for e in range(E):
            idx = slab_idx[e]
            if len(idx):
                out[idx] += res.results[e]["yt"][:, :len(idx)].T

    return out.reshape(B, S, H), logits


# revision 34
# speedup vs baseline: 1.0109x; 1.0109x over previous
"""Mixtral-style sparse MoE block on 8 Trainium2 NeuronCores.

Strategy (expert parallelism, per the sharding hint):
- The tiny gate MLP + softmax + top-2 routing runs on the host in fp64
  (routing decisions must match the fp32 reference; fp64 is the closest
  approximation to the true value and ~9 GFLOP is sub-second on host).
- Tokens are dispatched (gathered) per expert on the host; each of the 8
  NeuronCores holds exactly one expert's weights and runs the expert FFN
  y = (relu(x@w1) * (x@w3)) @ w2 for its gathered tokens.
- The top-2 combine weight is folded into the tokens before dispatch as
  sqrt(w): relu is positively homogeneous and the gated product is
  quadratic, so scaling x by sqrt(w) scales the FFN output by exactly w.
  This makes the device kernel a pure dense FFN with zero extra work.
- Matmuls run as float32r (fp32 with an 11-bit mantissa, processed at
  1 cycle/row by the PE vs 4 for full fp32). Operands are pre-rounded to
  f32r on the host; accumulation is fp32 in PSUM.
- Everything on the device is laid out transposed (feature dim on the
  128 SBUF partitions, tokens on the free axis) so all three matmuls use
  the weights as the stationary operand with no on-device transposes.
"""

import os

import numpy as np

import concourse.bacc as bacc
import concourse.mybir as mybir
import concourse.tile as tile
from concourse.bass_utils import run_bass_kernel_spmd

E = 8          # experts == cores
TOP_K = 2
H = 1024       # embed
F = 2048       # ffn
KO = H // 128  # 8 contraction chunks for H
NF = F // 128  # 16 f chunks
CMAX = 1280    # max tokens per expert per launch (SBUF budget)

F32 = mybir.dt.float32
F32R = mybir.dt.float32r

_PROGRAM_CACHE: dict[int, object] = {}
LAST_RESULTS = None  # BassKernelResults of the expert launch (for test harness)


def _ensure_profile_hook():
    """run_bass_kernel_spmd's trace path (BASS_TRACE=1) imports
    antenv.axon_hooks, which slim agent images lack. If it's missing,
    recreate the hook from trn_agent_boot's ctypes implementation and stub
    the artifact upload, so profiling works instead of crashing."""
    try:
        import antenv.axon_hooks  # noqa: F401
        return
    except ImportError:
        pass
    try:
        import sys
        import types

        import concourse.bass_utils as bu
        from trn_agent_boot.trn_boot import _ntff_profile_via_ctypes

        hook = _ntff_profile_via_ctypes("/opt/axon/libaxon_pjrt.so")
        mod = types.ModuleType("antenv.axon_hooks")
        mod.get_axon_ntff_profile_hook = lambda: hook
        sys.modules["antenv.axon_hooks"] = mod
        bu.upload_artifacts = lambda tmpdir: tmpdir
    except Exception:
        os.environ["BASS_NEVER_TRACE"] = "1"  # degrade: run untraced


def _round_f32r(x: np.ndarray) -> np.ndarray:
    """Round fp32 to f32r (round-to-nearest-even to 11 mantissa bits,
    low 12 bits zeroed) — matches what the PE requires for f32r operands."""
    u = np.ascontiguousarray(x, dtype=np.float32).view(np.uint32)
    lsb = (u >> np.uint32(12)) & np.uint32(1)
    r = (u + np.uint32(0x7FF) + lsb) & np.uint32(0xFFFFF000)
    return r.view(np.float32)


def _token_tiles(c: int):
    """Split the padded token dim into matmul tiles of 512 (tail >= 256).

    512-wide tiles keep the PE streaming above the ~190ns implicit f32r
    weight-load shadow; f32r also needs a moving dim >= 256 for full rate
    (even 448-wide tiles measured slower — weight-load-bound), so `c` is
    kept a multiple of 256 and only the tail tile drops to 256.
    """
    tiles = []
    c0 = 0
    while c0 < c:
        cw = 512 if c - c0 >= 512 else c - c0
        tiles.append((c0, cw))
        c0 += cw
    return tiles


def _build_expert_program(c: int):
    """Bass program: per-core expert FFN for `c` (transposed) tokens.

    Inputs (per core, all f32r pre-rounded on host):
      xt [H, c]             tokens, transposed, pre-scaled by sqrt(combine)
      w1 [NF, 128, KO, 128] w1 rearranged: [f, ki, ko, fi] = w1[ko*128+ki, f*128+fi]
      w3 [NF, 128, KO, 128] same layout as w1
      w2 [KO, 128, NF, 128] [h, fi, fo, hc] = w2[fo*128+fi, h*128+hc]
    Output:
      yt [H, c] fp32        transposed expert output (already combine-weighted)
    """
    nc = bacc.Bacc("TRN2", target_bir_lowering=False, debug=False, num_devices=E)
    xtd = nc.declare_dram_parameter("xt", [H, c], F32R, isOutput=False)
    w1d = nc.declare_dram_parameter("w1", [NF, 128, KO, 128], F32R, isOutput=False)
    w3d = nc.declare_dram_parameter("w3", [NF, 128, KO, 128], F32R, isOutput=False)
    w2d = nc.declare_dram_parameter("w2", [KO, 128, NF, 128], F32R, isOutput=False)
    ytd = nc.declare_dram_parameter("yt", [H, c], F32, isOutput=True)

    ttiles = _token_tiles(c)

    with tile.TileContext(nc) as tc:
        with (
            tc.tile_pool(name="xtp", bufs=1) as xtp,
            tc.tile_pool(name="wp", bufs=2) as wp,
            tc.tile_pool(name="hp", bufs=1) as hp,
            tc.tile_pool(name="op", bufs=4) as op,
            tc.tile_pool(name="pp", bufs=2, space="PSUM") as pp,
        ):
            # The first f-chunk's weights are queued before the xt stream so
            # the PE's first accumulation group isn't stuck behind the whole
            # 5MB token load; xt itself loads per (token-tile, k) so early
            # matmuls only depend on the slices they actually read.
            w1t0 = wp.tile([128, KO, 128], F32R, tag="w1t", name="w1t0")
            w3t0 = wp.tile([128, KO, 128], F32R, tag="w3t", name="w3t0")
            xts = [xtp.tile([128, c], F32R, tag=f"xt{k}", name=f"xt{k}")
                   for k in range(KO)]
            # Phase A walks token tiles narrowest-first: the first matmul
            # group then only needs w1[0] + the smallest xt slice set.
            ttiles_a = sorted(ttiles, key=lambda t: t[1])
            nc.sync.dma_start(out=w1t0[:], in_=w1d[0])
            for ti, (c0, cw) in enumerate(ttiles_a):
                for k in range(KO):
                    nc.sync.dma_start(
                        out=xts[k][:, c0:c0 + cw],
                        in_=xtd[k * 128:(k + 1) * 128, c0:c0 + cw],
                    )
                if ti == 0:
                    nc.sync.dma_start(out=w3t0[:], in_=w3d[0])

            hts = [
                hp.tile([128, c], F32R, tag=f"ht{f}", name=f"ht{f}")
                for f in range(NF)
            ]

            # Phase A: hT[f] = relu(w1[f]^T @ xt) * (w3[f]^T @ xt)
            for f in range(NF):
                if f == 0:
                    w1t, w3t = w1t0, w3t0
                else:
                    w1t = wp.tile([128, KO, 128], F32R, tag="w1t", name=f"w1t{f}")
                    nc.sync.dma_start(out=w1t[:], in_=w1d[f])
                    w3t = wp.tile([128, KO, 128], F32R, tag="w3t", name=f"w3t{f}")
                    nc.sync.dma_start(out=w3t[:], in_=w3d[f])
                for (c0, cw) in ttiles_a:
                    p1 = pp.tile([128, 512], F32, tag="p1", name="p1")
                    for k in range(KO):
                        nc.tensor.matmul(
                            p1[:, :cw], w1t[:, k, :], xts[k][:, c0:c0 + cw],
                            start=(k == 0), stop=(k == KO - 1),
                        )
                    p3 = pp.tile([128, 512], F32, tag="p3", name="p3")
                    for k in range(KO):
                        nc.tensor.matmul(
                            p3[:, :cw], w3t[:, k, :], xts[k][:, c0:c0 + cw],
                            start=(k == 0), stop=(k == KO - 1),
                        )
                    rl = op.tile([128, 512], F32, tag="rl", name="rl")
                    nc.scalar.activation(
                        rl[:, :cw], p1[:, :cw], mybir.ActivationFunctionType.Relu
                    )
                    nc.vector.tensor_mul(
                        hts[f][:, c0:c0 + cw], rl[:, :cw], p3[:, :cw]
                    )

            # Phase B: yt[h] = sum_f w2[h][f]^T @ hT[f]
            for h in range(KO):
                w2t = wp.tile([128, NF, 128], F32R, tag="w2t", name=f"w2t{h}")
                nc.sync.dma_start(out=w2t[:], in_=w2d[h])
                for (c0, cw) in ttiles:
                    po = pp.tile([128, 512], F32, tag="po", name="po")
                    for f in range(NF):
                        nc.tensor.matmul(
                            po[:, :cw], w2t[:, f, :], hts[f][:, c0:c0 + cw],
                            start=(f == 0), stop=(f == NF - 1),
                        )
                    yo = op.tile([128, 512], F32, tag="yo", name="yo")
                    nc.vector.tensor_copy(yo[:, :cw], po[:, :cw])
                    nc.sync.dma_start(
                        out=ytd[h * 128:(h + 1) * 128, c0:c0 + cw], in_=yo[:, :cw]
                    )

    nc.compile()
    return nc


def _gate_and_route(x, gw1, gb1, gw2, gb2):
    """Host gate MLP (fp64) + softmax + top-2 routing."""
    g = x.astype(np.float64) @ gw1.astype(np.float64) + gb1.astype(np.float64)
    g = np.where(g > 0.0, g, np.expm1(np.minimum(g, 0.0)))
    logits = g @ gw2.astype(np.float64) + gb2.astype(np.float64)
    z = logits - logits.max(axis=-1, keepdims=True)
    p = np.exp(z)
    p /= p.sum(axis=-1, keepdims=True)
    sel = np.argsort(-p, axis=-1, kind="stable")[:, :TOP_K]  # ties: lowest idx
    rw = np.take_along_axis(p, sel, axis=-1)
    rw = rw / rw.sum(axis=-1, keepdims=True)
    return logits.astype(np.float32), sel, rw


def kernel(hidden_states, gw1, gb1, gw2, gb2, w1, w3, w2):
    global LAST_RESULTS
    _ensure_profile_hook()
    hidden_states = np.asarray(hidden_states, dtype=np.float32)
    gw1 = np.asarray(gw1, dtype=np.float32)
    gb1 = np.asarray(gb1, dtype=np.float32)
    gw2 = np.asarray(gw2, dtype=np.float32)
    gb2 = np.asarray(gb2, dtype=np.float32)
    w1 = np.asarray(w1, dtype=np.float32)
    w3 = np.asarray(w3, dtype=np.float32)
    w2 = np.asarray(w2, dtype=np.float32)

    B, S, _ = hidden_states.shape
    x = hidden_states.reshape(-1, H)
    t = x.shape[0]

    logits, sel, rw = _gate_and_route(x, gw1, gb1, gw2, gb2)

    # Dispatch: token indices + sqrt(combine) scale per expert.
    idx_e, scl_e = [], []
    for e in range(E):
        tok, slot = np.nonzero(sel == e)
        idx_e.append(tok)
        scl_e.append(np.sqrt(rw[tok, slot]).astype(np.float32))
    counts = np.array([len(i) for i in idx_e])

    # Uniform capacity (SPMD: same program on all 8 cores), multiple of 256.
    max_cnt = int(counts.max())
    n_slabs = max(1, -(-max_cnt // CMAX))  # ceil
    slab_cnt = -(-max_cnt // n_slabs)
    c = max(512, -(-slab_cnt // 256) * 256)

    if c not in _PROGRAM_CACHE:
        _PROGRAM_CACHE[c] = _build_expert_program(c)
    nc = _PROGRAM_CACHE[c]

    # Per-expert weight repack (f32r, layouts documented in _build_expert_program)
    w1r = _round_f32r(
        np.ascontiguousarray(w1.reshape(E, KO, 128, NF, 128).transpose(0, 3, 2, 1, 4))
    )
    w3r = _round_f32r(
        np.ascontiguousarray(w3.reshape(E, KO, 128, NF, 128).transpose(0, 3, 2, 1, 4))
    )
    w2r = _round_f32r(
        np.ascontiguousarray(w2.reshape(E, NF, 128, KO, 128).transpose(0, 3, 2, 1, 4))
    )

    out = np.zeros((t, H), dtype=np.float32)
    for slab in range(n_slabs):
        in_maps = []
        slab_idx = []
        for e in range(E):
            lo, hi = slab * slab_cnt, min((slab + 1) * slab_cnt, counts[e])
            idx = idx_e[e][lo:hi] if hi > lo else idx_e[e][:0]
            slab_idx.append(idx)
            xg = x[idx] * scl_e[e][lo:hi, None]  # [n, H]
            xt = np.zeros((H, c), dtype=np.float32)
            xt[:, :len(idx)] = xg.T
            in_maps.append({
                "xt": _round_f32r(xt),
                "w1": w1r[e],
                "w3": w3r[e],
                "w2": w2r[e],
            })
        res = run_bass_kernel_spmd(nc, in_maps, core_ids=list(range(E)))
        LAST_RESULTS = res
        for e in range(E):
            idx = slab_idx[e]
            if len(idx):
                out[idx] += res.results[e]["yt"][:, :len(idx)].T

    return out.reshape(B, S, H), logits
